# revision 5
# baseline (speedup 1.0000x reference)
"""CLIF spiking-neuron recurrence kernel for 8 Trainium2 NeuronCores.

Reference semantics (per element, T=64 sequential steps, gamma=0.5):
    u     = 0.5*u + x_t
    spike = (u >= 1.0)
    m     = s_prev * sigmoid(0.5*u) + spike
    s     = sigmoid(m)                       # carried (in-place sigmoid_)
    u     = u - spike*(1.0 + s)
Output: spikes [T, B, D] float32.

Strategy:
- Pure data parallel over the B*D = 524288 elements: 65536 per core as
  [128 partitions x 512 free], split into G=2 independent pipeline
  groups of [128 x 256] so one group's serial dependency chain hides
  under the other group's engine work.
- The membrane potential lives in PSUM as V_t = 2^t * u_t (power-of-2
  scaling is exact in fp32; 2^63*|u|max is far below fp32 range), so
  the leak (x0.5) costs nothing: it folds into per-step constants.
- Per step and group the engines do:
    ACT:  sg = sigmoid(2^-(t+1) * V)  and  s = sigmoid(M)
    DVE:  q  = s_prev * sg                         (tensor_tensor)
          M  = (sg >= c) + q                       (scalar_tensor_tensor)
          zn = -2^t * s - 2^t                      (tensor_scalar, 2x mode)
          y  = (sg >= c) * zn  [= -2^t*spike*(1+s)] (scalar_tensor_tensor)
    DMA:  y += 2^(t+1) * x_(t+1)   (software-DGE accumulate-on-transfer)
    PE:   V += I @ y               (one fp32 identity matmul)
  where c = sigmoidLUT(0.5) is computed on-device once; the ACT LUT is
  strictly monotone around z=0.5 (verified on HW), so (sg >= c) <=>
  (u >= 1) exactly.
- The kernel streams out sg (fp32); the host applies spike = (sg >= c),
  bit-identical to the on-device compares.
"""

import numpy as np
import ml_dtypes
import concourse.bass as bass
import concourse.bacc as bacc
import concourse.mybir as mybir
import concourse.tile as tile
from concourse.bass_utils import run_bass_kernel_spmd

F32 = mybir.dt.float32
AF = mybir.ActivationFunctionType
ALU = mybir.AluOpType

T = 64
B = 128
D = 4096
N_CORES = 8
P = 128
NPC = B * D // N_CORES          # 65536 elements per core
FDT = NPC // P                  # 512 free columns per core
G = 2                           # pipeline groups
FD = FDT // G                   # 256 free columns per group

_NC_CACHE = None
LAST_RESULTS = None


def _build():
    nc = bacc.Bacc(None, target_bir_lowering=False, debug=False,
                   num_devices=N_CORES)

    xs = nc.declare_dram_parameter("xs", [T, G, P, FD], F32, isOutput=False)
    wt = nc.declare_dram_parameter("wt", [P, P], F32, isOutput=False)  # identity
    out = nc.declare_dram_parameter("out", [T, G, P, FD], F32, isOutput=True)
    cout = nc.declare_dram_parameter("cout", [P, 1], F32, isOutput=True)

    with tile.TileContext(nc) as tc:
        with (
            tc.tile_pool(name="wpool", bufs=1) as wpool,
            tc.tile_pool(name="cpool", bufs=1) as cpool,
            tc.tile_pool(name="sgpool", bufs=4) as sgpool,
            tc.tile_pool(name="spool", bufs=3) as spool,
            tc.tile_pool(name="qpool", bufs=2) as qpool,
            tc.tile_pool(name="mpool", bufs=2) as mpool,
            tc.tile_pool(name="zpool", bufs=2) as zpool,
            tc.tile_pool(name="ypool", bufs=4) as ypool,
            tc.tile_pool(name="vpool", bufs=1, space="PSUM") as vpool,
        ):
            # --- one-time setup -------------------------------------------
            eye = wpool.tile([P, P], F32, tag="eye")
            nc.sync.dma_start(eye[:], wt[:])

            halft = cpool.tile([P, 1], F32, tag="half")
            nc.gpsimd.memset(halft[:], 0.5)
            ct = cpool.tile([P, 1], F32, tag="c")
            # c = sigmoid_LUT(0.5), same LUT as the per-step sigmoids
            nc.scalar.activation(ct[:], halft[:], AF.Sigmoid, bias=0.0, scale=1.0)
            nc.sync.dma_start(cout[:], ct[:])
            c_ap = ct[:, 0:1]

            # --- initial state --------------------------------------------
            V = []
            s_prev = []
            for g in range(G):
                s0 = spool.tile([P, FD], F32, tag=f"s{g}")
                nc.gpsimd.memset(s0[:], 0.0)
                s_prev.append(s0)
                vt = vpool.tile([P, FD], F32, tag=f"V{g}")
                V.append(vt)

            for g in range(G):
                y0 = ypool.tile([P, FD], F32, tag=f"y{g}")
                nc.sync.dma_start(y0[:], xs[0, g])
                nc.tensor.matmul(V[g][:], eye[:], y0[:], start=True, stop=False,
                                 skip_group_check=True)

            # --- the recurrence -------------------------------------------
            for t in range(T):
                sc_sg = float(2.0 ** (-t - 1))
                zneg = float(-(2.0 ** t))
                for g in range(G):
                    # sg = sigmoid(0.5 * u_t); streamed out, thresholded on host
                    sg = sgpool.tile([P, FD], F32, tag=f"sg{g}")
                    nc.scalar.activation(sg[:], V[g][:], AF.Sigmoid,
                                         bias=0.0, scale=sc_sg)
                    nc.sync.dma_start(out[t, g], sg[:])

                    if t == T - 1:
                        continue  # state updates past the last step are dead

                    # q = s_prev * sg ; M = (sg >= c) + q ; s = sigmoid(M)
                    q = qpool.tile([P, FD], F32, tag=f"q{g}")
                    nc.vector.tensor_mul(q[:], s_prev[g][:], sg[:])
                    msb = mpool.tile([P, FD], F32, tag=f"m{g}")
                    nc.vector.scalar_tensor_tensor(
                        msb[:], sg[:], c_ap, q[:], op0=ALU.is_ge, op1=ALU.add)
                    s_new = spool.tile([P, FD], F32, tag=f"s{g}")
                    nc.scalar.activation(s_new[:], msb[:], AF.Sigmoid,
                                         bias=0.0, scale=1.0)
                    s_prev[g] = s_new

                    # y = (sg >= c) * (-2^t * (1 + s)) + 2^(t+1) x_(t+1)
                    zn = zpool.tile([P, FD], F32, tag=f"z{g}")
                    nc.vector.tensor_scalar(zn[:], s_new[:], zneg, zneg,
                                            op0=ALU.mult, op1=ALU.add)
                    y = ypool.tile([P, FD], F32, tag=f"y{g}")
                    nc.vector.scalar_tensor_tensor(
                        y[:], sg[:], c_ap, zn[:], op0=ALU.is_ge, op1=ALU.mult)
                    nc.gpsimd.dma_start(y[:], xs[t + 1, g], accum_op=ALU.add)

                    # V += I @ y
                    nc.tensor.matmul(V[g][:], eye[:], y[:],
                                     start=False, stop=(t + 1 == T - 1),
                                     skip_group_check=True)

    nc.compile()
    return nc


def _get_nc():
    global _NC_CACHE
    if _NC_CACHE is None:
        _NC_CACHE = _build()
    return _NC_CACHE


def kernel(x_seq: np.ndarray) -> np.ndarray:
    global LAST_RESULTS
    x = np.ascontiguousarray(x_seq, dtype=np.float32)
    assert x.shape == (T, B, D), x.shape

    # 2^t prescale (exact in fp32) and per-core shard [T, G, P, FD]
    scale = (2.0 ** np.arange(T, dtype=np.float64)).astype(np.float32)
    xsc = x.reshape(T, -1) * scale[:, None]
    xsc = xsc.reshape(T, N_CORES, G, P, FD)

    eye_host = np.eye(P, dtype=np.float32)

    nc = _get_nc()
    in_maps = [
        {"xs": np.ascontiguousarray(xsc[:, c]), "wt": eye_host}
        for c in range(N_CORES)
    ]
    LAST_RESULTS = run_bass_kernel_spmd(nc, in_maps, list(range(N_CORES)))

    full = np.empty((T, N_CORES, G, P, FD), dtype=np.float32)
    for c in range(N_CORES):
        res = LAST_RESULTS.results[c]
        c_val = np.asarray(res["cout"], dtype=np.float32)[0, 0]
        sg = np.asarray(res["out"], dtype=np.float32)
        full[:, c] = (sg >= c_val).astype(np.float32)
    return full.reshape(T, B, D)


# revision 6
# speedup vs baseline: 1.9808x; 1.9808x over previous
"""CLIF spiking-neuron recurrence kernel for 8 Trainium2 NeuronCores.

Reference semantics (per element, T=64 sequential steps, gamma=0.5):
    u     = 0.5*u + x_t
    spike = (u >= 1.0)
    m     = s_prev * sigmoid(0.5*u) + spike
    s     = sigmoid(m)                       # carried (in-place sigmoid_)
    u     = u - spike*(1.0 + s)
Output: spikes [T, B, D] float32.

Strategy:
- Pure data parallel over the B*D = 524288 elements: 65536 per core as
  [128 partitions x 512 free], split into G=2 independent pipeline
  groups of [128 x 256] so one group's serial dependency chain hides
  under the other group's engine work.
- The membrane potential lives in PSUM as V_t = 2^t * u_t (power-of-2
  scaling is exact in fp32; 2^63*|u|max is far below fp32 range), so
  the leak (x0.5) folds into per-step constants and the input add runs
  on the TensorEngine as an fp32 identity-matmul accumulate.
- Two custom DVE ops (registered into the per-NEFF uop table, verified
  bit-exact on HW) fuse the whole elementwise step into 2-3 Vector ops:
    CLIF_M: M = s_prev*sg + (sg >= c)            (q-mult + spike + add)
    CLIF_Y: y = (sg >= c) * (s*(-2^t) + (-2^t))  [= -2^t*spike*(1+s)]
  where sg = sigmoid(2^-(t+1) * V) and c = sigmoidLUT(0.5) is computed
  on-device once; the ACT LUT is strictly monotone around z=0.5
  (verified on HW), so (sg >= c) <=> (u >= 1) exactly.
- Per step+group: ACT runs the two sigmoids; DVE runs CLIF_M/CLIF_Y;
  the V update V += y + 2^(t+1)*x runs as one matmul (group 0, with the
  x-add folded into y on DVE) or two matmuls (group 1) to balance DVE
  against the TensorEngine.
- The kernel streams out sg (fp32); the host applies spike = (sg >= c),
  bit-identical to the on-device compares.
"""

import numpy as np
import ml_dtypes
import concourse.bass as bass
import concourse.bacc as bacc
import concourse.mybir as mybir
import concourse.tile as tile
import concourse.dve_ops as dve_ops
from concourse.dve_spec import Spec, Src0, Src1, C0, C1, lower, _has_src1
from concourse.dve_uop import DveOpSpec
from concourse.bass_utils import run_bass_kernel_spmd

F32 = mybir.dt.float32
AF = mybir.ActivationFunctionType
ALU = mybir.AluOpType

T = 64
B = 128
D = 4096
N_CORES = 8
P = 128
NPC = B * D // N_CORES          # 65536 elements per core
FDT = NPC // P                  # 512 free columns per core
G = 2                           # pipeline groups
FD = FDT // G                   # 256 free columns per group

_NC_CACHE = None
LAST_RESULTS = None


def _register_dve_op(name, spec):
    for op in dve_ops.OPS:
        if op.name == name:
            return op
    shas = {}
    for ver in ("v3", "v4"):
        u = lower(spec, ver=ver)
        shas[ver] = DveOpSpec(name=name, opcode=1, uops=u,
                              rd1_en=_has_src1(spec)).sha(ver)
    op = dve_ops.DveOp(name, spec, subdim=False, uops_sha=shas)
    dve_ops.OPS.append(op)
    dve_ops._SUB_OPCODE_FOR_NAME[name] = (
        dve_ops._CUSTOM_DVE_ROW_BASE + len(dve_ops.OPS) - 1)
    dve_ops.CUSTOM_DVE_SPECS[name] = spec
    return op


# M = s_prev*sg + (sg >= c)          in0=s_prev, in1=sg, s0=c
CLIF_M = _register_dve_op("CLIF_M_ANT", Spec(
    body=Src0 * Src1 + (Src1 >= C0),
    reference=lambda in0, in1, s0, s1, imm2:
        in0 * in1 + (in1 >= s0).astype(np.float32),
))
# y = (sg >= c) * (s*zneg + zneg)    in0=s, in1=sg, s0=c, s1=zneg=-2^t
CLIF_Y = _register_dve_op("CLIF_Y_ANT", Spec(
    body=(Src1 >= C0) * (Src0 * C1 + C1),
    reference=lambda in0, in1, s0, s1, imm2:
        (in1 >= s0).astype(np.float32) * (in0 * s1 + s1),
))


def _build():
    nc = bacc.Bacc(None, target_bir_lowering=False, debug=False,
                   num_devices=N_CORES)

    xs = nc.declare_dram_parameter("xs", [T, G, P, FD], F32, isOutput=False)
    wt = nc.declare_dram_parameter("wt", [P, P], F32, isOutput=False)  # identity
    out = nc.declare_dram_parameter("out", [T, G, P, FD], F32, isOutput=True)
    cout = nc.declare_dram_parameter("cout", [P, 1], F32, isOutput=True)

    with tile.TileContext(nc) as tc:
        with (
            tc.tile_pool(name="wpool", bufs=1) as wpool,
            tc.tile_pool(name="cpool", bufs=1) as cpool,
            tc.tile_pool(name="xpool", bufs=6) as xpool,
            tc.tile_pool(name="sgpool", bufs=4) as sgpool,
            tc.tile_pool(name="spool", bufs=3) as spool,
            tc.tile_pool(name="mpool", bufs=2) as mpool,
            tc.tile_pool(name="ypool", bufs=3) as ypool,
            tc.tile_pool(name="vpool", bufs=1, space="PSUM") as vpool,
        ):
            # --- one-time setup -------------------------------------------
            eye = wpool.tile([P, P], F32, tag="eye")
            nc.sync.dma_start(eye[:], wt[:])

            halft = cpool.tile([P, 1], F32, tag="half")
            nc.gpsimd.memset(halft[:], 0.5)
            ct = cpool.tile([P, 1], F32, tag="c")
            # c = sigmoid_LUT(0.5), same LUT as the per-step sigmoids
            nc.scalar.activation(ct[:], halft[:], AF.Sigmoid, bias=0.0, scale=1.0)
            nc.sync.dma_start(cout[:], ct[:])
            c_ap = ct[:, 0:1]

            # --- initial state --------------------------------------------
            V = []
            s_prev = []
            for g in range(G):
                s0 = spool.tile([P, FD], F32, tag=f"s{g}")
                nc.gpsimd.memset(s0[:], 0.0)
                s_prev.append(s0)
                vt = vpool.tile([P, FD], F32, tag=f"V{g}")
                V.append(vt)

            for g in range(G):
                x0 = xpool.tile([P, FD], F32, tag=f"x{g}")
                nc.sync.dma_start(x0[:], xs[0, g])
                nc.tensor.matmul(V[g][:], eye[:], x0[:], start=True, stop=False,
                                 skip_group_check=True)

            # --- the recurrence -------------------------------------------
            for t in range(T):
                sc_sg = float(2.0 ** (-t - 1))
                zneg = float(-(2.0 ** t))
                for g in range(G):
                    # sg = sigmoid(0.5 * u_t); streamed out, host thresholds
                    sg = sgpool.tile([P, FD], F32, tag=f"sg{g}")
                    nc.scalar.activation(sg[:], V[g][:], AF.Sigmoid,
                                         bias=0.0, scale=sc_sg)
                    nc.sync.dma_start(out[t, g], sg[:])

                    if t == T - 1:
                        continue  # state updates past the last step are dead

                    # M = s_prev*sg + spike ; s = sigmoid(M)
                    msb = mpool.tile([P, FD], F32, tag=f"m{g}")
                    nc.vector._custom_dve(CLIF_M, out=msb[:], in0=s_prev[g][:],
                                          in1=sg[:], s0=c_ap)
                    s_new = spool.tile([P, FD], F32, tag=f"s{g}")
                    nc.scalar.activation(s_new[:], msb[:], AF.Sigmoid,
                                         bias=0.0, scale=1.0)
                    s_prev[g] = s_new

                    # y = -2^t * spike * (1 + s)
                    y = ypool.tile([P, FD], F32, tag=f"y{g}")
                    nc.vector._custom_dve(CLIF_Y, out=y[:], in0=s_new[:],
                                          in1=sg[:], s0=c_ap, s1=zneg)

                    xt = xpool.tile([P, FD], F32, tag=f"x{g}")
                    nc.sync.dma_start(xt[:], xs[t + 1, g])
                    last = (t + 1 == T - 1)
                    if g == 0:
                        # fold the x-add into y on DVE; single matmul
                        nc.vector.tensor_add(y[:], y[:], xt[:])
                        nc.tensor.matmul(V[g][:], eye[:], y[:],
                                         start=False, stop=last,
                                         skip_group_check=True)
                    else:
                        nc.tensor.matmul(V[g][:], eye[:], y[:],
                                         start=False, stop=False,
                                         skip_group_check=True)
                        nc.tensor.matmul(V[g][:], eye[:], xt[:],
                                         start=False, stop=last,
                                         skip_group_check=True)

    nc.compile()
    return nc


def _get_nc():
    global _NC_CACHE
    if _NC_CACHE is None:
        _NC_CACHE = _build()
    return _NC_CACHE


def kernel(x_seq: np.ndarray) -> np.ndarray:
    global LAST_RESULTS
    x = np.ascontiguousarray(x_seq, dtype=np.float32)
    assert x.shape == (T, B, D), x.shape

    # 2^t prescale (exact in fp32) and per-core shard [T, G, P, FD]
    scale = (2.0 ** np.arange(T, dtype=np.float64)).astype(np.float32)
    xsc = x.reshape(T, -1) * scale[:, None]
    xsc = xsc.reshape(T, N_CORES, G, P, FD)

    eye_host = np.eye(P, dtype=np.float32)

    nc = _get_nc()
    in_maps = [
        {"xs": np.ascontiguousarray(xsc[:, c]), "wt": eye_host}
        for c in range(N_CORES)
    ]
    LAST_RESULTS = run_bass_kernel_spmd(nc, in_maps, list(range(N_CORES)))

    full = np.empty((T, N_CORES, G, P, FD), dtype=np.float32)
    for c in range(N_CORES):
        res = LAST_RESULTS.results[c]
        c_val = np.asarray(res["cout"], dtype=np.float32)[0, 0]
        sg = np.asarray(res["out"], dtype=np.float32)
        full[:, c] = (sg >= c_val).astype(np.float32)
    return full.reshape(T, B, D)


# revision 8
# speedup vs baseline: 2.0749x; 1.0475x over previous
"""CLIF spiking-neuron recurrence kernel for 8 Trainium2 NeuronCores.

Reference semantics (per element, T=64 sequential steps, gamma=0.5):
    u     = 0.5*u + x_t
    spike = (u >= 1.0)
    m     = s_prev * sigmoid(0.5*u) + spike
    s     = sigmoid(m)                       # carried (in-place sigmoid_)
    u     = u - spike*(1.0 + s)
Output: spikes [T, B, D] float32.

Strategy:
- Pure data parallel over the B*D = 524288 elements: 65536 per core as
  [128 partitions x 512 free], split into G=2 independent pipeline
  groups of [128 x 256] so one group's serial dependency chain hides
  under the other group's engine work.
- The membrane potential lives in PSUM as V_t = 2^t * u_t (power-of-2
  scaling is exact in fp32; 2^63*|u|max is far below fp32 range), so
  the leak (x0.5) folds into per-step constants and the input add runs
  on the TensorEngine as an fp32 identity-matmul accumulate.
- Two custom DVE ops (registered into the per-NEFF uop table, verified
  bit-exact on HW) fuse the whole elementwise step into 2-3 Vector ops:
    CLIF_M: M = s_prev*sg + (sg >= c)            (q-mult + spike + add)
    CLIF_Y: y = (sg >= c) * (s*(-2^t) + (-2^t))  [= -2^t*spike*(1+s)]
  where sg = sigmoid(2^-(t+1) * V) and c = sigmoidLUT(0.5) is computed
  on-device once; the ACT LUT is strictly monotone around z=0.5
  (verified on HW), so (sg >= c) <=> (u >= 1) exactly.
- Per step+group: ACT runs the two sigmoids; DVE runs CLIF_M/CLIF_Y;
  the V update V += y + 2^(t+1)*x runs as one matmul (group 0, with the
  x-add folded into y on DVE) or two matmuls (group 1) to balance DVE
  against the TensorEngine.
- The kernel streams out sg (fp32); the host applies spike = (sg >= c),
  bit-identical to the on-device compares.
"""

import numpy as np
import ml_dtypes
import concourse.bass as bass
import concourse.bacc as bacc
import concourse.mybir as mybir
import concourse.tile as tile
import concourse.dve_ops as dve_ops
from concourse.dve_spec import Spec, Src0, Src1, C0, C1, lower, _has_src1
from concourse.dve_uop import DveOpSpec
from concourse.bass_utils import run_bass_kernel_spmd

F32 = mybir.dt.float32
AF = mybir.ActivationFunctionType
ALU = mybir.AluOpType

T = 64
B = 128
D = 4096
N_CORES = 8
P = 128
NPC = B * D // N_CORES          # 65536 elements per core
FDT = NPC // P                  # 512 free columns per core
G = 2                           # pipeline groups
FD = FDT // G                   # 256 free columns per group

_NC_CACHE = None
LAST_RESULTS = None


def _register_dve_op(name, spec):
    for op in dve_ops.OPS:
        if op.name == name:
            return op
    shas = {}
    for ver in ("v3", "v4"):
        u = lower(spec, ver=ver)
        shas[ver] = DveOpSpec(name=name, opcode=1, uops=u,
                              rd1_en=_has_src1(spec)).sha(ver)
    op = dve_ops.DveOp(name, spec, subdim=False, uops_sha=shas)
    dve_ops.OPS.append(op)
    dve_ops._SUB_OPCODE_FOR_NAME[name] = (
        dve_ops._CUSTOM_DVE_ROW_BASE + len(dve_ops.OPS) - 1)
    dve_ops.CUSTOM_DVE_SPECS[name] = spec
    return op


# M = s_prev*sg + (sg >= c)          in0=s_prev, in1=sg, s0=c
CLIF_M = _register_dve_op("CLIF_M_ANT", Spec(
    body=Src0 * Src1 + (Src1 >= C0),
    reference=lambda in0, in1, s0, s1, imm2:
        in0 * in1 + (in1 >= s0).astype(np.float32),
))
# y = (sg >= c) * (s*zneg + zneg)    in0=s, in1=sg, s0=c, s1=zneg=-2^t
CLIF_Y = _register_dve_op("CLIF_Y_ANT", Spec(
    body=(Src1 >= C0) * (Src0 * C1 + C1),
    reference=lambda in0, in1, s0, s1, imm2:
        (in1 >= s0).astype(np.float32) * (in0 * s1 + s1),
))


def _build():
    nc = bacc.Bacc(None, target_bir_lowering=False, debug=False,
                   num_devices=N_CORES)

    xs = nc.declare_dram_parameter("xs", [T, P, FDT], F32, isOutput=False)
    wt = nc.declare_dram_parameter("wt", [P, P], F32, isOutput=False)  # identity
    out = nc.declare_dram_parameter("out", [T, P, FDT], F32, isOutput=True)
    cout = nc.declare_dram_parameter("cout", [P, 1], F32, isOutput=True)

    with tile.TileContext(nc) as tc:
        with (
            tc.tile_pool(name="wpool", bufs=1) as wpool,
            tc.tile_pool(name="cpool", bufs=1) as cpool,
            tc.tile_pool(name="xpool", bufs=6) as xpool,
            tc.tile_pool(name="sgpool", bufs=4) as sgpool,
            tc.tile_pool(name="spool", bufs=3) as spool,
            tc.tile_pool(name="mpool", bufs=2) as mpool,
            tc.tile_pool(name="ypool", bufs=3) as ypool,
            tc.tile_pool(name="vpool", bufs=1, space="PSUM") as vpool,
        ):
            # --- one-time setup -------------------------------------------
            eye = wpool.tile([P, P], F32, tag="eye")
            nc.sync.dma_start(eye[:], wt[:])

            halft = cpool.tile([P, 1], F32, tag="half")
            nc.gpsimd.memset(halft[:], 0.5)
            ct = cpool.tile([P, 1], F32, tag="c")
            # c = sigmoid_LUT(0.5), same LUT as the per-step sigmoids
            nc.scalar.activation(ct[:], halft[:], AF.Sigmoid, bias=0.0, scale=1.0)
            nc.sync.dma_start(cout[:], ct[:])
            c_ap = ct[:, 0:1]

            # --- initial state --------------------------------------------
            V = []
            s_prev = []
            for g in range(G):
                s0 = spool.tile([P, FD], F32, tag=f"s{g}")
                nc.gpsimd.memset(s0[:], 0.0)
                s_prev.append(s0)
                vt = vpool.tile([P, FD], F32, tag=f"V{g}")
                V.append(vt)

            x0 = xpool.tile([P, G * FD], F32, tag="x")
            nc.sync.dma_start(x0[:], xs[0])
            for g in range(G):
                nc.tensor.matmul(V[g][:], eye[:], x0[:, g * FD:(g + 1) * FD],
                                 start=True, stop=False, skip_group_check=True)
            xcur = x0

            # --- the recurrence -------------------------------------------
            for t in range(T):
                sc_sg = float(2.0 ** (-t - 1))
                zneg = float(-(2.0 ** t))

                # one wide input prefetch per step (both groups)
                if t < T - 1:
                    xnext = xpool.tile([P, G * FD], F32, tag="x")
                    nc.sync.dma_start(xnext[:], xs[t + 1])

                # one wide output tile per step; ACT fills per-group halves
                sgw = sgpool.tile([P, G * FD], F32, tag="sg")
                for g in range(G):
                    sg = sgw[:, g * FD:(g + 1) * FD]
                    nc.scalar.activation(sg, V[g][:], AF.Sigmoid,
                                         bias=0.0, scale=sc_sg)

                    if t == T - 1:
                        continue  # state updates past the last step are dead

                    # M = s_prev*sg + spike ; s = sigmoid(M)
                    msb = mpool.tile([P, FD], F32, tag=f"m{g}")
                    nc.vector._custom_dve(CLIF_M, out=msb[:], in0=s_prev[g][:],
                                          in1=sg, s0=c_ap)
                    s_new = spool.tile([P, FD], F32, tag=f"s{g}")
                    nc.scalar.activation(s_new[:], msb[:], AF.Sigmoid,
                                         bias=0.0, scale=1.0)
                    s_prev[g] = s_new

                    # y = -2^t*spike*(1+s), then y += 2^(t+1)*x_(t+1) on DVE
                    y = ypool.tile([P, FD], F32, tag=f"y{g}")
                    nc.vector._custom_dve(CLIF_Y, out=y[:], in0=s_new[:],
                                          in1=sg, s0=c_ap, s1=zneg)
                    nc.vector.tensor_add(y[:], y[:],
                                         xnext[:, g * FD:(g + 1) * FD])
                    nc.tensor.matmul(V[g][:], eye[:], y[:],
                                     start=False, stop=(t + 1 == T - 1),
                                     skip_group_check=True)

                nc.sync.dma_start(out[t], sgw[:])
                if t < T - 1:
                    xcur = xnext

    nc.compile()
    return nc


def _get_nc():
    global _NC_CACHE
    if _NC_CACHE is None:
        _NC_CACHE = _build()
    return _NC_CACHE


def kernel(x_seq: np.ndarray) -> np.ndarray:
    global LAST_RESULTS
    x = np.ascontiguousarray(x_seq, dtype=np.float32)
    assert x.shape == (T, B, D), x.shape

    # 2^t prescale (exact in fp32) and per-core shard [T, P, FDT]
    scale = (2.0 ** np.arange(T, dtype=np.float64)).astype(np.float32)
    xsc = x.reshape(T, -1) * scale[:, None]
    xsc = xsc.reshape(T, N_CORES, P, FDT)

    eye_host = np.eye(P, dtype=np.float32)

    nc = _get_nc()
    in_maps = [
        {"xs": np.ascontiguousarray(xsc[:, c]), "wt": eye_host}
        for c in range(N_CORES)
    ]
    LAST_RESULTS = run_bass_kernel_spmd(nc, in_maps, list(range(N_CORES)))

    full = np.empty((T, N_CORES, P, FDT), dtype=np.float32)
    for c in range(N_CORES):
        res = LAST_RESULTS.results[c]
        c_val = np.asarray(res["cout"], dtype=np.float32)[0, 0]
        sg = np.asarray(res["out"], dtype=np.float32)
        full[:, c] = (sg >= c_val).astype(np.float32)
    return full.reshape(T, B, D)


# revision 9
# speedup vs baseline: 3.2289x; 1.5561x over previous
"""CLIF spiking-neuron recurrence kernel for 8 Trainium2 NeuronCores.

Reference semantics (per element, T=64 sequential steps, gamma=0.5):
    u     = 0.5*u + x_t
    spike = (u >= 1.0)
    m     = s_prev * sigmoid(0.5*u) + spike
    s     = sigmoid(m)                       # carried (in-place sigmoid_)
    u     = u - spike*(1.0 + s)
Output: spikes [T, B, D] float32.

Strategy:
- Pure data parallel over the B*D = 524288 elements: 65536 per core as
  [128 partitions x 512 free], split into G independent pipeline groups
  along the free dim. Each group's step is a serial dependency loop
  (sigmoid -> CLIF_M -> sigmoid -> CLIF_Y -> matmul); with the input
  matmul hoisted off that loop, the kernel is latency-bound at
  T * loop-latency, and groups overlap on the engines.
- The membrane potential lives in PSUM as V_t = 2^t * u_t (power-of-2
  scaling is exact in fp32; 2^63*|u|max is far below fp32 range). The
  leak folds into per-step constants; the input add V += I @ (2^t x_t)
  runs on the TensorEngine right after step t-1's sigmoid read, off the
  critical loop; the reset matmul V += I @ y closes the loop.
- Two custom DVE ops (registered into the per-NEFF uop table, verified
  bit-exact on HW) fuse all elementwise work into 2 Vector ops:
    CLIF_M: M = s_prev*sg + (sg >= c)            (q-mult + spike + add)
    CLIF_Y: y = (sg >= c) * (s*(-2^t) + (-2^t))  [= -2^t*spike*(1+s)]
  where sg = sigmoid(2^-(t+1) * V) and c = sigmoidLUT(0.5) is computed
  on-device once; the ACT LUT is strictly monotone around z=0.5
  (verified on HW), so (sg >= c) <=> (u >= 1) exactly.
- One wide [128,512] input DMA and one wide output DMA per step.
- The kernel streams out sg (fp32); the host applies spike = (sg >= c),
  bit-identical to the on-device compares.
"""

import numpy as np
import ml_dtypes
import concourse.bass as bass
import concourse.bacc as bacc
import concourse.mybir as mybir
import concourse.tile as tile
import concourse.dve_ops as dve_ops
from concourse.dve_spec import Spec, Src0, Src1, C0, C1, lower, _has_src1
from concourse.dve_uop import DveOpSpec
from concourse.bass_utils import run_bass_kernel_spmd

F32 = mybir.dt.float32
AF = mybir.ActivationFunctionType
ALU = mybir.AluOpType

T = 64
B = 128
D = 4096
N_CORES = 8
P = 128
NPC = B * D // N_CORES          # 65536 elements per core
FDT = NPC // P                  # 512 free columns per core

# group column ranges (start, width) along the 512-wide free dim
GROUPS = [(0, 176), (176, 176), (352, 160)]

_NC_CACHE = None
LAST_RESULTS = None


def _register_dve_op(name, spec):
    for op in dve_ops.OPS:
        if op.name == name:
            return op
    shas = {}
    for ver in ("v3", "v4"):
        u = lower(spec, ver=ver)
        shas[ver] = DveOpSpec(name=name, opcode=1, uops=u,
                              rd1_en=_has_src1(spec)).sha(ver)
    op = dve_ops.DveOp(name, spec, subdim=False, uops_sha=shas)
    dve_ops.OPS.append(op)
    dve_ops._SUB_OPCODE_FOR_NAME[name] = (
        dve_ops._CUSTOM_DVE_ROW_BASE + len(dve_ops.OPS) - 1)
    dve_ops.CUSTOM_DVE_SPECS[name] = spec
    return op


# M = s_prev*sg + (sg >= c)          in0=s_prev, in1=sg, s0=c
CLIF_M = _register_dve_op("CLIF_M_ANT", Spec(
    body=Src0 * Src1 + (Src1 >= C0),
    reference=lambda in0, in1, s0, s1, imm2:
        in0 * in1 + (in1 >= s0).astype(np.float32),
))
# y = (sg >= c) * (s*zneg + zneg)    in0=s, in1=sg, s0=c, s1=zneg=-2^t
CLIF_Y = _register_dve_op("CLIF_Y_ANT", Spec(
    body=(Src1 >= C0) * (Src0 * C1 + C1),
    reference=lambda in0, in1, s0, s1, imm2:
        (in1 >= s0).astype(np.float32) * (in0 * s1 + s1),
))


def _build():
    nc = bacc.Bacc(None, target_bir_lowering=False, debug=False,
                   num_devices=N_CORES)

    xs = nc.declare_dram_parameter("xs", [T, P, FDT], F32, isOutput=False)
    wt = nc.declare_dram_parameter("wt", [P, P], F32, isOutput=False)  # identity
    out = nc.declare_dram_parameter("out", [T, P, FDT], F32, isOutput=True)
    cout = nc.declare_dram_parameter("cout", [P, 1], F32, isOutput=True)

    G = len(GROUPS)
    with tile.TileContext(nc) as tc:
        with (
            tc.tile_pool(name="wpool", bufs=1) as wpool,
            tc.tile_pool(name="cpool", bufs=1) as cpool,
            tc.tile_pool(name="xpool", bufs=6) as xpool,
            tc.tile_pool(name="sgpool", bufs=5) as sgpool,
            tc.tile_pool(name="spool", bufs=3) as spool,
            tc.tile_pool(name="mpool", bufs=3) as mpool,
            tc.tile_pool(name="ypool", bufs=3) as ypool,
            tc.tile_pool(name="vpool", bufs=1, space="PSUM") as vpool,
        ):
            # --- one-time setup -------------------------------------------
            eye = wpool.tile([P, P], F32, tag="eye")
            nc.sync.dma_start(eye[:], wt[:])

            halft = cpool.tile([P, 1], F32, tag="half")
            nc.gpsimd.memset(halft[:], 0.5)
            ct = cpool.tile([P, 1], F32, tag="c")
            # c = sigmoid_LUT(0.5), same LUT as the per-step sigmoids
            nc.scalar.activation(ct[:], halft[:], AF.Sigmoid, bias=0.0, scale=1.0)
            nc.sync.dma_start(cout[:], ct[:])
            c_ap = ct[:, 0:1]

            # --- initial state --------------------------------------------
            V = []
            s_prev = []
            for g, (o, w) in enumerate(GROUPS):
                s0 = spool.tile([P, w], F32, tag=f"s{g}")
                nc.gpsimd.memset(s0[:], 0.0)
                s_prev.append(s0)
                vt = vpool.tile([P, w], F32, tag=f"V{g}")
                V.append(vt)

            x0 = xpool.tile([P, FDT], F32, tag="x")
            nc.sync.dma_start(x0[:], xs[0])
            for g, (o, w) in enumerate(GROUPS):
                nc.tensor.matmul(V[g][:], eye[:], x0[:, o:o + w],
                                 start=True, stop=False, skip_group_check=True)

            # --- the recurrence -------------------------------------------
            for t in range(T):
                sc_sg = float(2.0 ** (-t - 1))
                zneg = float(-(2.0 ** t))

                # one wide input prefetch per step (all groups)
                if t < T - 1:
                    xnext = xpool.tile([P, FDT], F32, tag="x")
                    nc.sync.dma_start(xnext[:], xs[t + 1])

                # one wide output tile per step; ACT fills per-group slices
                sgw = sgpool.tile([P, FDT], F32, tag="sg")
                for g, (o, w) in enumerate(GROUPS):
                    sg = sgw[:, o:o + w]
                    nc.scalar.activation(sg, V[g][:], AF.Sigmoid,
                                         bias=0.0, scale=sc_sg)

                    if t == T - 1:
                        continue  # state updates past the last step are dead

                    # input add for the NEXT step: off the critical loop,
                    # legal as soon as this step's sigmoid has read V
                    nc.tensor.matmul(V[g][:], eye[:], xnext[:, o:o + w],
                                     start=False, stop=False,
                                     skip_group_check=True)

                    # M = s_prev*sg + spike ; s = sigmoid(M)
                    msb = mpool.tile([P, w], F32, tag=f"m{g}")
                    nc.vector._custom_dve(CLIF_M, out=msb[:], in0=s_prev[g][:],
                                          in1=sg, s0=c_ap)
                    s_new = spool.tile([P, w], F32, tag=f"s{g}")
                    nc.scalar.activation(s_new[:], msb[:], AF.Sigmoid,
                                         bias=0.0, scale=1.0)
                    s_prev[g] = s_new

                    # y = -2^t * spike * (1+s) ; V += I @ y closes the loop
                    y = ypool.tile([P, w], F32, tag=f"y{g}")
                    nc.vector._custom_dve(CLIF_Y, out=y[:], in0=s_new[:],
                                          in1=sg, s0=c_ap, s1=zneg)
                    nc.tensor.matmul(V[g][:], eye[:], y[:],
                                     start=False, stop=(t + 1 == T - 1),
                                     skip_group_check=True)

                nc.sync.dma_start(out[t], sgw[:])

    nc.compile()
    return nc


def _get_nc():
    global _NC_CACHE
    if _NC_CACHE is None:
        _NC_CACHE = _build()
    return _NC_CACHE


def kernel(x_seq: np.ndarray) -> np.ndarray:
    global LAST_RESULTS
    x = np.ascontiguousarray(x_seq, dtype=np.float32)
    assert x.shape == (T, B, D), x.shape

    # 2^t prescale (exact in fp32) and per-core shard [T, P, FDT]
    scale = (2.0 ** np.arange(T, dtype=np.float64)).astype(np.float32)
    xsc = x.reshape(T, -1) * scale[:, None]
    xsc = xsc.reshape(T, N_CORES, P, FDT)

    eye_host = np.eye(P, dtype=np.float32)

    nc = _get_nc()
    in_maps = [
        {"xs": np.ascontiguousarray(xsc[:, c]), "wt": eye_host}
        for c in range(N_CORES)
    ]
    LAST_RESULTS = run_bass_kernel_spmd(nc, in_maps, list(range(N_CORES)))

    full = np.empty((T, N_CORES, P, FDT), dtype=np.float32)
    for c in range(N_CORES):
        res = LAST_RESULTS.results[c]
        c_val = np.asarray(res["cout"], dtype=np.float32)[0, 0]
        sg = np.asarray(res["out"], dtype=np.float32)
        full[:, c] = (sg >= c_val).astype(np.float32)
    return full.reshape(T, B, D)
